# revision 24
# baseline (speedup 1.0000x reference)
"""Single-head self-attention over 8192 assets (D=512) on 8 TRN2 NeuronCores.

Sharding (sequence-parallel over the asset/row dim):
  - core i owns query rows [i*1024, (i+1)*1024)
  - each core computes qT/kT/v projections for its own 1024 rows (bf16)
  - the kT/v shards are shared via EIGHT split AllGathers (128 tokens x
    8 ranks each) that run on the TOPSP/SDMA silicon, pipelined against
    the attention compute
  - each core processes its OWN block first straight out of SBUF (filling
    the window where the CC stream sits in its all-core entry barrier),
    then streams the gathered slices of the 7 REMOTE ranks via per-core
    rotated dynamic-offset DMAs, accumulating its 1024 rows of
    softmax(q k^T / sqrt(D)) @ v

Kernel-level choices:
  - matmul operands are bf16 (fp22 multiply path, fp32 PSUM accumulate);
    biases, accumulators and the output stay fp32
  - scores are computed TRANSPOSED: scoresT[n, m] = k @ q^T, so that
    exp(scoresT) feeds the attention matmul directly as the stationary
    operand (no attention-matrix transpose anywhere)
  - softmax needs no max-subtraction: |scores/sqrt(D)| < ~3 for this
    problem family (z ~ N(0,1), W ~ U(+-1/sqrt(D))), exp is safe in fp32
  - softmax denominator: a ones-stationary matmul per score chunk writes
    the denominator replicated across partitions into its own PSUM bank
    (PSUM start=True zeroes a whole 2KB bank, so groups cannot share one);
    32x32 DVE block-transposes turn the replicated rows into per-partition
    columns for the final normalization
  - biases: bq/bk are per-partition ACT biases fused into the PSUM->SBUF
    copy; bv is a rank-1 (ones x bv) matmul seeded into PSUM first
  - the host pre-transposes z and the (tiny) weight matrices and casts
    them to bf16, so no on-chip transposes or rounding copies are needed
"""

import numpy as np
import ml_dtypes

import concourse.mybir as mybir
from concourse.bass import _add_dep_helper as bass_dep, ds as bass_ds
import concourse.tile as tile
from concourse import bacc
from concourse.bass_utils import run_bass_kernel_spmd

N_CORES = 8
N_TOK = 8192
D = 512
M_LOC = N_TOK // N_CORES   # 1024 query rows per core / tokens per kv shard
P = 128                    # SBUF partitions
DC = D // P                # 4 chunks of the latent dim
MB = M_LOC // 512          # 2 m-blocks of 512 queries
NQ = 8                     # split gathers
QTOK = M_LOC // NQ         # tokens per rank per gather slice
QT = QTOK // P             # 2 key chunks of 128 per rank per quarter
SCALE = float(1.0 / np.sqrt(D))

F32 = mybir.dt.float32
BF16 = mybir.dt.bfloat16

KT_Q = D * QTOK            # kT quarter elems per rank
V_Q = QTOK * D             # v quarter elems per rank
KV_Q = KT_Q + V_Q


def _build():
    nc = bacc.Bacc("TRN2", target_bir_lowering=False, debug=False,
                   num_devices=N_CORES)

    zT_d = nc.dram_tensor("zT_loc", [D, M_LOC], BF16, kind="ExternalInput")
    WqT_d = nc.dram_tensor("WqT", [D, D], BF16, kind="ExternalInput")
    WkT_d = nc.dram_tensor("WkT", [D, D], BF16, kind="ExternalInput")
    WvT_d = nc.dram_tensor("WvT", [D, D], BF16, kind="ExternalInput")
    bq_d = nc.dram_tensor("bq", [D], F32, kind="ExternalInput")
    bk_d = nc.dram_tensor("bk", [D], F32, kind="ExternalInput")
    bv_d = nc.dram_tensor("bv", [1, D], BF16, kind="ExternalInput")
    ones_row_d = nc.dram_tensor("ones_row", [1, P], BF16, kind="ExternalInput")
    ones_sq_d = nc.dram_tensor("ones_sq", [P, P], BF16, kind="ExternalInput")

    h_d = nc.dram_tensor("h_out", [M_LOC, D], F32, kind="ExternalOutput")

    kv_in = [nc.dram_tensor(f"kv_in{a}", [KV_Q], BF16) for a in range(NQ)]
    kv_all = [nc.dram_tensor(f"kv_all{a}", [N_CORES * KV_Q], BF16,
                             addr_space="Shared") for a in range(NQ)]
    offs_d = nc.dram_tensor("offs", [1, 2 * (N_CORES - 1)], mybir.dt.int32,
                            kind="ExternalInput")

    def kt_view(flat):
        return flat[0:KT_Q].rearrange("(p c m) -> p c m", p=P, c=DC)

    def v_view(flat):
        return flat[KT_Q:KV_Q].rearrange("(p t d) -> p t d", p=P, t=QT)

    with tile.TileContext(nc) as tc:
        with (
            tc.tile_pool(name="const", bufs=1) as const,
            tc.tile_pool(name="persist", bufs=1) as persist,
        ):
            # ---- constants / weights ----
            WqT_sb = const.tile([P, DC, D], BF16)
            WkT_sb = const.tile([P, DC, D], BF16)
            WvT_sb = const.tile([P, DC, D], BF16)
            for sb, dr in ((WkT_sb, WkT_d), (WvT_sb, WvT_d), (WqT_sb, WqT_d)):
                nc.sync.dma_start(sb[:], dr.ap().rearrange("(c p) d -> p c d", p=P))
            bq_sb = const.tile([P, DC], F32)
            bk_sb = const.tile([P, DC], F32)
            bv_sb = const.tile([1, D], BF16)
            ones_row = const.tile([1, P], BF16)
            ones_sq = const.tile([P, P], BF16)
            zeros_col = const.tile([P, 1], F32)
            nc.sync.dma_start(bk_sb[:], bk_d.ap().rearrange("(c p) -> p c", p=P))
            nc.sync.dma_start(bq_sb[:], bq_d.ap().rearrange("(c p) -> p c", p=P))
            nc.sync.dma_start(bv_sb[:], bv_d[:])
            nc.sync.dma_start(ones_row[:], ones_row_d[:])
            nc.sync.dma_start(ones_sq[:], ones_sq_d[:])
            nc.gpsimd.memset(zeros_col[:], 0.0)

            qT_sb = persist.tile([P, DC, M_LOC], BF16)
            kTl_sb = persist.tile([P, DC, M_LOC], BF16)
            vl_sb = persist.tile([P, MB * 4, D], BF16)
            h_acc = persist.tile([P, MB * 4, D], F32)
            den_acc = persist.tile([P, MB, 512], F32)
            offs_sb = persist.tile([1, 2 * (N_CORES - 1)], mybir.dt.int32)
            nc.sync.dma_start(offs_sb[:], offs_d[:])

            cc_insts = []

            # ---- projections for the core's own 1024 rows ----
            with (
                tc.tile_pool(name="proj", bufs=1) as proj,
                tc.tile_pool(name="ps_proj", bufs=2, space="PSUM") as ps_proj,
            ):
                zT_sb = proj.tile([P, DC, M_LOC], BF16)
                zT_dv = zT_d.ap().rearrange("(c p) m -> p c m", p=P)
                nc.sync.dma_start(zT_sb[:, :, 0:512], zT_dv[:, :, 0:512])
                nc.sync.dma_start(zT_sb[:, :, 512:M_LOC], zT_dv[:, :, 512:M_LOC])

                # k/v projections interleaved with the quarter bounces so
                # each gather fires as soon as its 256-token slice is ready;
                # the q projection runs under the gathers
                def k_proj(mb):
                    for dc in range(DC):
                        ps = ps_proj.tile([P, 512], F32, name="ps")
                        for c in range(DC):
                            nc.tensor.matmul(
                                ps[:],
                                WkT_sb[:, c, dc * P:(dc + 1) * P],
                                zT_sb[:, c, mb * 512:(mb + 1) * 512],
                                start=(c == 0), stop=(c == DC - 1),
                            )
                        nc.scalar.activation(
                            kTl_sb[:, dc, mb * 512:(mb + 1) * 512], ps[:],
                            mybir.ActivationFunctionType.Identity,
                            bias=bk_sb[:, dc:dc + 1],
                        )

                def v_proj(t):
                    ps = ps_proj.tile([P, 512], F32, name="ps")
                    nc.tensor.matmul(ps[:], ones_row[:], bv_sb[:],
                                     start=True, stop=False)
                    for c in range(DC):
                        nc.tensor.matmul(
                            ps[:],
                            zT_sb[:, c, t * P:(t + 1) * P],
                            WvT_sb[:, c, :],
                            start=False, stop=(c == DC - 1),
                        )
                    nc.scalar.copy(vl_sb[:, t, :], ps[:])

                def bounce(a):
                    nc.sync.dma_start(kt_view(kv_in[a].ap()),
                                      kTl_sb[:, :, a * QTOK:(a + 1) * QTOK])
                    nc.sync.dma_start(v_view(kv_in[a].ap()),
                                      vl_sb[:, a * QT:(a + 1) * QT, :])
                    cc = nc.gpsimd.collective_compute(
                        "AllGather",
                        mybir.AluOpType.bypass,
                        replica_groups=[list(range(N_CORES))],
                        ins=[kv_in[a].ap().opt()],
                        outs=[kv_all[a].ap().opt()],
                    )
                    cc_insts.append(cc)

                # interleave so gather a fires as soon as its kT (needs the
                # right k-proj half) and v slice are projected
                half = NQ // MB  # gathers per k-proj half
                for mbk in range(MB):
                    k_proj(mbk)
                    for a in range(mbk * half, (mbk + 1) * half):
                        for t in range(a * QT, (a + 1) * QT):
                            v_proj(t)
                        bounce(a)

                for dc in range(DC):
                    for mb in range(MB):
                        ps = ps_proj.tile([P, 512], F32, name="ps")
                        for c in range(DC):
                            nc.tensor.matmul(
                                ps[:],
                                WqT_sb[:, c, dc * P:(dc + 1) * P],
                                zT_sb[:, c, mb * 512:(mb + 1) * 512],
                                start=(c == 0), stop=(c == DC - 1),
                            )
                        nc.scalar.activation(
                            qT_sb[:, dc, mb * 512:(mb + 1) * 512], ps[:],
                            mybir.ActivationFunctionType.Identity,
                            bias=bq_sb[:, dc:dc + 1],
                        )

            # ---- attention ----
            # The core's OWN 1024-token block runs first straight out of
            # SBUF (it needs no collective, so it fills the window where the
            # CC stream is still in its all-core entry barrier). The
            # gathered slices then cover only the 7 REMOTE ranks, read
            # from kv_all via per-core rotated dynamic offsets.
            NR = N_CORES - 1
            NCH_R = NR * QT  # remote chunks per gather slice
            kt_rv = [nc.values_load(offs_sb[0:1, j:j + 1],
                                    engines={mybir.EngineType.SP})
                     for j in range(NR)]
            v_rv = [nc.values_load(offs_sb[0:1, NR + j:NR + j + 1],
                                   engines={mybir.EngineType.SP})
                    for j in range(NR)]
            with (
                tc.tile_pool(name="blk", bufs=2) as blk,
                tc.tile_pool(name="pTp", bufs=6) as pTp,
                tc.tile_pool(name="pairp", bufs=3) as pairp,
                tc.tile_pool(name="ps_s", bufs=2, space="PSUM") as ps_s,
                tc.tile_pool(name="ps_h", bufs=5, space="PSUM") as ps_h,
                tc.tile_pool(name="ps_den", bufs=1, space="PSUM") as ps_den,
            ):
                pending = []  # one-step software pipeline: PE runs the next
                              # scores group while ACT exps the previous
                prev_pT = []

                def flush_pending():
                    pT, hs, dn, v_ap, start, stop, drain, mb = pending.pop()
                    for mt in range(4):
                        nc.tensor.matmul(
                            hs[mt][:],
                            pT[:, mt * P:(mt + 1) * P],
                            v_ap,
                            start=start, stop=stop,
                        )
                    # denominator once per chunk PAIR: DVE sums the two exp
                    # tiles, then a ones-stationary matmul accumulates it
                    # (replicated across partitions) into the den bank
                    if not prev_pT:
                        prev_pT.append((pT, start))
                    else:
                        p0, start0 = prev_pT.pop()
                        pr = pairp.tile([P, 512], BF16, name="pr")
                        nc.vector.tensor_add(pr[:], p0[:], pT[:])
                        nc.tensor.matmul(
                            dn[:],
                            ones_sq[:],
                            pr[:],
                            start=start0, stop=stop,
                        )
                    if stop and prev_pT:
                        p0, start0 = prev_pT.pop()
                        nc.tensor.matmul(
                            dn[:],
                            ones_sq[:],
                            p0[:],
                            start=start0, stop=True,
                        )
                    if drain is not None:
                        for mt in range(4):
                            j = mb * 4 + mt
                            if drain == "copy":
                                nc.vector.tensor_copy(h_acc[:, j, :], hs[mt][:])
                            else:
                                nc.vector.tensor_add(
                                    h_acc[:, j, :], hs[mt][:], h_acc[:, j, :])
                        sl = den_acc[:, mb, :]
                        if drain == "copy":
                            nc.vector.tensor_copy(sl, dn[:])
                        else:
                            nc.vector.tensor_add(sl, dn[:], sl)

                def emit_set(kt_at, v_at, nch, drain_kind):
                    # one full sweep: for each m-block, scores+exp+attention
                    # over nch key chunks, accumulated in PSUM then drained
                    for mb in range(MB):
                        hs = [ps_h.tile([P, D], F32, name=f"h{mt}", tag="hps")
                              for mt in range(4)]
                        dn = ps_den.tile([P, 512], F32, name="dn")
                        for u in range(nch):
                            ps = ps_s.tile([P, 512], F32, name="ps_sc")
                            for c in range(DC):
                                nc.tensor.matmul(
                                    ps[:],
                                    kt_at(c, u),
                                    qT_sb[:, c, mb * 512:(mb + 1) * 512],
                                    start=(c == 0), stop=(c == DC - 1),
                                )
                            if pending:
                                flush_pending()
                            pT = pTp.tile([P, 512], BF16, name="pT")
                            nc.scalar.activation(
                                pT[:], ps[:],
                                mybir.ActivationFunctionType.Exp,
                                bias=zeros_col[:], scale=SCALE,
                            )
                            pending.append(
                                (pT, hs, dn, v_at(u), u == 0, u == nch - 1,
                                 drain_kind if u == nch - 1 else None, mb))

                # own block from SBUF: no collective dependency
                emit_set(lambda c, u: kTl_sb[:, c, u * P:(u + 1) * P],
                         lambda u: vl_sb[:, u, :], MB * 4, "copy")

                for a in range(NQ):
                    kT_q = blk.tile([P, DC, NR * QTOK], BF16, name="kT_q")
                    v_q = blk.tile([P, NCH_R, D], BF16, name="v_q")
                    for j in range(NR):
                        d1 = nc.sync.dma_start(
                            kT_q[:, :, j * QTOK:(j + 1) * QTOK],
                            kv_all[a].ap()[bass_ds(kt_rv[j], KT_Q)]
                            .rearrange("(p c m) -> p c m", p=P, c=DC))
                        d2 = nc.sync.dma_start(
                            v_q[:, j * QT:(j + 1) * QT, :],
                            kv_all[a].ap()[bass_ds(v_rv[j], V_Q)]
                            .rearrange("(p t d) -> p t d", p=P, t=QT))
                        # dynamic-offset APs are not region-tracked against
                        # the collective's write; order them explicitly
                        for dd in (d1, d2):
                            bass_dep(dd.ins, cc_insts[a].ins, sync=True,
                                     reason="dyn kv read after gather")
                    emit_set(lambda c, u, kT_q=kT_q: kT_q[:, c, u * P:(u + 1) * P],
                             lambda u, v_q=v_q: v_q[:, u, :], NCH_R, "add")
                flush_pending()

                # ---- normalize and write out (per m-block, pipelined) ----
                rcpw = persist.tile([P, MB, 512], F32)
                scr = persist.tile([P, MB * 4 * 32], F32)
                h_dv = h_d.ap().rearrange("(t p) d -> p t d", p=P)
                for mb in range(MB):
                    for mt in range(4):
                        j = mb * 4 + mt
                        for x in range(4):
                            nc.vector.transpose(
                                scr[32 * x:32 * x + 32, j * 32:(j + 1) * 32],
                                den_acc[32 * x:32 * x + 32, mb,
                                        mt * P + 32 * x:mt * P + 32 * x + 32])
                        nc.vector.reciprocal(rcpw[:, mb, mt:mt + 1],
                                             scr[:, j * 32:j * 32 + 1])
                        nc.vector.tensor_scalar_mul(
                            h_acc[:, j, :], h_acc[:, j, :],
                            rcpw[:, mb, mt:mt + 1])
                        nc.sync.dma_start(h_dv[:, j, :], h_acc[:, j, :])

    nc.compile()
    return nc


_cache = {}


def kernel(z, Wq, bq, Wk, bk, Wv, bv):
    if "nc" not in _cache:
        _cache["nc"] = _build()
    nc = _cache["nc"]

    bf16 = ml_dtypes.bfloat16
    z, Wq, bq, Wk, bk, Wv, bv = (np.asarray(t) for t in
                                 (z, Wq, bq, Wk, bk, Wv, bv))
    z = np.ascontiguousarray(z, dtype=np.float32)
    zT = np.ascontiguousarray(z.T).astype(bf16)
    base = {
        "WqT": np.ascontiguousarray(Wq.T).astype(bf16),
        "WkT": np.ascontiguousarray(Wk.T).astype(bf16),
        "WvT": np.ascontiguousarray(Wv.T).astype(bf16),
        "bq": np.ascontiguousarray(bq, dtype=np.float32),
        "bk": np.ascontiguousarray(bk, dtype=np.float32),
        "bv": np.ascontiguousarray(bv).astype(bf16).reshape(1, D),
        "ones_row": np.ones((1, P), dtype=bf16),
        "ones_sq": np.ones((P, P), dtype=bf16),
    }
    in_maps = []
    for i in range(N_CORES):
        m = dict(base)
        m["zT_loc"] = np.ascontiguousarray(zT[:, i * M_LOC:(i + 1) * M_LOC])
        rem = [((i + 1 + j) % N_CORES) * KV_Q for j in range(N_CORES - 1)]
        m["offs"] = np.array([rem + [r + KT_Q for r in rem]], dtype=np.int32)
        in_maps.append(m)

    _cache["in_maps"] = in_maps
    res = run_bass_kernel_spmd(nc, in_maps, core_ids=list(range(N_CORES)))
    _cache["last_result"] = res
    return np.concatenate(
        [res.results[i]["h_out"] for i in range(N_CORES)], axis=0)



# revision 25
# speedup vs baseline: 1.0380x; 1.0380x over previous
"""Single-head self-attention over 8192 assets (D=512) on 8 TRN2 NeuronCores.

Sharding (sequence-parallel over the asset/row dim):
  - core i owns query rows [i*1024, (i+1)*1024)
  - each core computes qT/kT/v projections for its own 1024 rows (bf16)
  - the kT/v shards are shared via EIGHT split AllGathers (128 tokens x
    8 ranks each) that run on the TOPSP/SDMA silicon, pipelined against
    the attention compute
  - each core processes its OWN block first straight out of SBUF (filling
    the window where the CC stream sits in its all-core entry barrier),
    then streams the gathered slices of the 7 REMOTE ranks via per-core
    rotated dynamic-offset DMAs, accumulating its 1024 rows of
    softmax(q k^T / sqrt(D)) @ v

Kernel-level choices:
  - matmul operands are bf16 (fp22 multiply path, fp32 PSUM accumulate);
    biases, accumulators and the output stay fp32
  - scores are computed TRANSPOSED: scoresT[n, m] = k @ q^T, so that
    exp(scoresT) feeds the attention matmul directly as the stationary
    operand (no attention-matrix transpose anywhere)
  - softmax needs no max-subtraction: |scores/sqrt(D)| < ~3 for this
    problem family (z ~ N(0,1), W ~ U(+-1/sqrt(D))), exp is safe in fp32
  - softmax denominator: a ones-stationary matmul per score chunk writes
    the denominator replicated across partitions into its own PSUM bank
    (PSUM start=True zeroes a whole 2KB bank, so groups cannot share one);
    32x32 DVE block-transposes turn the replicated rows into per-partition
    columns for the final normalization
  - biases: bq/bk are per-partition ACT biases fused into the PSUM->SBUF
    copy; bv is a rank-1 (ones x bv) matmul seeded into PSUM first
  - the host pre-transposes z and the (tiny) weight matrices and casts
    them to bf16, so no on-chip transposes or rounding copies are needed
"""

import numpy as np
import ml_dtypes

import concourse.mybir as mybir
from concourse.bass import _add_dep_helper as bass_dep, ds as bass_ds
import concourse.tile as tile
from concourse import bacc
from concourse.bass_utils import run_bass_kernel_spmd

N_CORES = 8
N_TOK = 8192
D = 512
M_LOC = N_TOK // N_CORES   # 1024 query rows per core / tokens per kv shard
P = 128                    # SBUF partitions
DC = D // P                # 4 chunks of the latent dim
MB = M_LOC // 512          # 2 m-blocks of 512 queries
NQ = 8                     # split gathers
QTOK = M_LOC // NQ         # tokens per rank per gather slice
QT = QTOK // P             # 2 key chunks of 128 per rank per quarter
SCALE = float(1.0 / np.sqrt(D))

F32 = mybir.dt.float32
BF16 = mybir.dt.bfloat16

KT_Q = D * QTOK            # kT quarter elems per rank
V_Q = QTOK * D             # v quarter elems per rank
KV_Q = KT_Q + V_Q


def _build():
    nc = bacc.Bacc("TRN2", target_bir_lowering=False, debug=False,
                   num_devices=N_CORES)

    zT_d = nc.dram_tensor("zT_loc", [D, M_LOC], BF16, kind="ExternalInput")
    WqT_d = nc.dram_tensor("WqT", [D, D], BF16, kind="ExternalInput")
    WkT_d = nc.dram_tensor("WkT", [D, D], BF16, kind="ExternalInput")
    WvT_d = nc.dram_tensor("WvT", [D, D], BF16, kind="ExternalInput")
    bq_d = nc.dram_tensor("bq", [D], F32, kind="ExternalInput")
    bk_d = nc.dram_tensor("bk", [D], F32, kind="ExternalInput")
    bv_d = nc.dram_tensor("bv", [1, D], BF16, kind="ExternalInput")
    ones_row_d = nc.dram_tensor("ones_row", [1, P], BF16, kind="ExternalInput")
    ones_sq_d = nc.dram_tensor("ones_sq", [P, P], BF16, kind="ExternalInput")

    h_d = nc.dram_tensor("h_out", [M_LOC, D], F32, kind="ExternalOutput")

    kv_in = [nc.dram_tensor(f"kv_in{a}", [KV_Q], BF16) for a in range(NQ)]
    kv_all = [nc.dram_tensor(f"kv_all{a}", [N_CORES * KV_Q], BF16,
                             addr_space="Shared") for a in range(NQ)]
    offs_d = nc.dram_tensor("offs", [1, 2 * (N_CORES - 1)], mybir.dt.int32,
                            kind="ExternalInput")

    def kt_view(flat):
        return flat[0:KT_Q].rearrange("(p c m) -> p c m", p=P, c=DC)

    def v_view(flat):
        return flat[KT_Q:KV_Q].rearrange("(p t d) -> p t d", p=P, t=QT)

    with tile.TileContext(nc) as tc:
        with (
            tc.tile_pool(name="const", bufs=1) as const,
            tc.tile_pool(name="persist", bufs=1) as persist,
        ):
            # ---- constants / weights ----
            WqT_sb = const.tile([P, DC, D], BF16)
            WkT_sb = const.tile([P, DC, D], BF16)
            WvT_sb = const.tile([P, DC, D], BF16)
            for sb, dr in ((WkT_sb, WkT_d), (WvT_sb, WvT_d), (WqT_sb, WqT_d)):
                nc.sync.dma_start(sb[:], dr.ap().rearrange("(c p) d -> p c d", p=P))
            bq_sb = const.tile([P, DC], F32)
            bk_sb = const.tile([P, DC], F32)
            bv_sb = const.tile([1, D], BF16)
            ones_row = const.tile([1, P], BF16)
            ones_sq = const.tile([P, P], BF16)
            zeros_col = const.tile([P, 1], F32)
            nc.sync.dma_start(bk_sb[:], bk_d.ap().rearrange("(c p) -> p c", p=P))
            nc.sync.dma_start(bq_sb[:], bq_d.ap().rearrange("(c p) -> p c", p=P))
            nc.sync.dma_start(bv_sb[:], bv_d[:])
            nc.sync.dma_start(ones_row[:], ones_row_d[:])
            nc.sync.dma_start(ones_sq[:], ones_sq_d[:])
            nc.gpsimd.memset(zeros_col[:], 0.0)

            qT_sb = persist.tile([P, DC, M_LOC], BF16)
            kTl_sb = persist.tile([P, DC, M_LOC], BF16)
            vl_sb = persist.tile([P, MB * 4, D], BF16)
            h_acc = persist.tile([P, MB * 4, D], F32)
            den_acc = persist.tile([P, MB, 512], F32)
            offs_sb = persist.tile([1, 2 * (N_CORES - 1)], mybir.dt.int32)
            nc.sync.dma_start(offs_sb[:], offs_d[:])

            cc_insts = []

            # ---- projections for the core's own 1024 rows ----
            with (
                tc.tile_pool(name="proj", bufs=1) as proj,
                tc.tile_pool(name="ps_proj", bufs=2, space="PSUM") as ps_proj,
            ):
                zT_sb = proj.tile([P, DC, M_LOC], BF16)
                zT_dv = zT_d.ap().rearrange("(c p) m -> p c m", p=P)
                nc.sync.dma_start(zT_sb[:, :, 0:512], zT_dv[:, :, 0:512])
                nc.sync.dma_start(zT_sb[:, :, 512:M_LOC], zT_dv[:, :, 512:M_LOC])

                # k/v projections interleaved with the quarter bounces so
                # each gather fires as soon as its 256-token slice is ready;
                # the q projection runs under the gathers
                def k_proj(mb):
                    for dc in range(DC):
                        ps = ps_proj.tile([P, 512], F32, name="ps")
                        for c in range(DC):
                            nc.tensor.matmul(
                                ps[:],
                                WkT_sb[:, c, dc * P:(dc + 1) * P],
                                zT_sb[:, c, mb * 512:(mb + 1) * 512],
                                start=(c == 0), stop=(c == DC - 1),
                            )
                        nc.scalar.activation(
                            kTl_sb[:, dc, mb * 512:(mb + 1) * 512], ps[:],
                            mybir.ActivationFunctionType.Identity,
                            bias=bk_sb[:, dc:dc + 1],
                        )

                def v_proj(t):
                    ps = ps_proj.tile([P, 512], F32, name="ps")
                    nc.tensor.matmul(ps[:], ones_row[:], bv_sb[:],
                                     start=True, stop=False)
                    for c in range(DC):
                        nc.tensor.matmul(
                            ps[:],
                            zT_sb[:, c, t * P:(t + 1) * P],
                            WvT_sb[:, c, :],
                            start=False, stop=(c == DC - 1),
                        )
                    nc.scalar.copy(vl_sb[:, t, :], ps[:])

                def bounce(a):
                    nc.sync.dma_start(kt_view(kv_in[a].ap()),
                                      kTl_sb[:, :, a * QTOK:(a + 1) * QTOK])
                    nc.sync.dma_start(v_view(kv_in[a].ap()),
                                      vl_sb[:, a * QT:(a + 1) * QT, :])
                    cc = nc.gpsimd.collective_compute(
                        "AllGather",
                        mybir.AluOpType.bypass,
                        replica_groups=[list(range(N_CORES))],
                        ins=[kv_in[a].ap().opt()],
                        outs=[kv_all[a].ap().opt()],
                    )
                    cc_insts.append(cc)

                # interleave so gather a fires as soon as its kT (needs the
                # right k-proj half) and v slice are projected
                half = NQ // MB  # gathers per k-proj half
                for mbk in range(MB):
                    k_proj(mbk)
                    for a in range(mbk * half, (mbk + 1) * half):
                        for t in range(a * QT, (a + 1) * QT):
                            v_proj(t)
                        bounce(a)

                for dc in range(DC):
                    for mb in range(MB):
                        ps = ps_proj.tile([P, 512], F32, name="ps")
                        for c in range(DC):
                            nc.tensor.matmul(
                                ps[:],
                                WqT_sb[:, c, dc * P:(dc + 1) * P],
                                zT_sb[:, c, mb * 512:(mb + 1) * 512],
                                start=(c == 0), stop=(c == DC - 1),
                            )
                        nc.scalar.activation(
                            qT_sb[:, dc, mb * 512:(mb + 1) * 512], ps[:],
                            mybir.ActivationFunctionType.Identity,
                            bias=bq_sb[:, dc:dc + 1],
                        )

            # ---- attention ----
            # The core's OWN 1024-token block runs first straight out of
            # SBUF (it needs no collective, so it fills the window where the
            # CC stream is still in its all-core entry barrier). The
            # gathered slices then cover only the 7 REMOTE ranks, read
            # from kv_all via per-core rotated dynamic offsets.
            NR = N_CORES - 1
            NCH_R = NR * QT  # remote chunks per gather slice
            kt_rv = [nc.values_load(offs_sb[0:1, j:j + 1],
                                    engines={mybir.EngineType.SP})
                     for j in range(NR)]
            v_rv = [nc.values_load(offs_sb[0:1, NR + j:NR + j + 1],
                                   engines={mybir.EngineType.SP})
                    for j in range(NR)]
            with (
                tc.tile_pool(name="blk", bufs=2) as blk,
                tc.tile_pool(name="pTp", bufs=6) as pTp,
                tc.tile_pool(name="pairp", bufs=3) as pairp,
                tc.tile_pool(name="ps_s", bufs=1, space="PSUM") as ps_s,
                tc.tile_pool(name="ps_h", bufs=6, space="PSUM") as ps_h,
                tc.tile_pool(name="ps_den", bufs=1, space="PSUM") as ps_den,
            ):
                pending = []  # one-step software pipeline: PE runs the next
                              # scores group while ACT exps the previous
                prev_pT = []

                def flush_pending():
                    pT, hs, dn, v_ap, start, stop, drain, mb = pending.pop()
                    for mt in range(4):
                        nc.tensor.matmul(
                            hs[mt][:],
                            pT[:, mt * P:(mt + 1) * P],
                            v_ap,
                            start=start, stop=stop,
                        )
                    # denominator once per chunk PAIR: DVE sums the two exp
                    # tiles, then a ones-stationary matmul accumulates it
                    # (replicated across partitions) into the den bank
                    if not prev_pT:
                        prev_pT.append((pT, start))
                    else:
                        p0, start0 = prev_pT.pop()
                        pr = pairp.tile([P, 512], BF16, name="pr")
                        nc.vector.tensor_add(pr[:], p0[:], pT[:])
                        nc.tensor.matmul(
                            dn[:],
                            ones_sq[:],
                            pr[:],
                            start=start0, stop=stop,
                        )
                    if stop and prev_pT:
                        p0, start0 = prev_pT.pop()
                        nc.tensor.matmul(
                            dn[:],
                            ones_sq[:],
                            p0[:],
                            start=start0, stop=True,
                        )
                    if drain is not None:
                        for mt in range(4):
                            j = mb * 4 + mt
                            if drain == "copy":
                                nc.vector.tensor_copy(h_acc[:, j, :], hs[mt][:])
                            else:
                                nc.vector.tensor_add(
                                    h_acc[:, j, :], hs[mt][:], h_acc[:, j, :])
                        sl = den_acc[:, mb, :]
                        if drain == "copy":
                            nc.vector.tensor_copy(sl, dn[:])
                        else:
                            nc.vector.tensor_add(sl, dn[:], sl)

                def emit_set(kt_at, v_at, nch, drain_kind):
                    # one full sweep: for each m-block, scores+exp+attention
                    # over nch key chunks, accumulated in PSUM then drained
                    for mb in range(MB):
                        hs = [ps_h.tile([P, D], F32, name=f"h{mt}", tag="hps")
                              for mt in range(4)]
                        dn = ps_den.tile([P, 512], F32, name="dn")
                        for u in range(nch):
                            ps = ps_s.tile([P, 512], F32, name="ps_sc")
                            for c in range(DC):
                                nc.tensor.matmul(
                                    ps[:],
                                    kt_at(c, u),
                                    qT_sb[:, c, mb * 512:(mb + 1) * 512],
                                    start=(c == 0), stop=(c == DC - 1),
                                )
                            if pending:
                                flush_pending()
                            pT = pTp.tile([P, 512], BF16, name="pT")
                            nc.scalar.activation(
                                pT[:], ps[:],
                                mybir.ActivationFunctionType.Exp,
                                bias=zeros_col[:], scale=SCALE,
                            )
                            pending.append(
                                (pT, hs, dn, v_at(u), u == 0, u == nch - 1,
                                 drain_kind if u == nch - 1 else None, mb))

                # own block from SBUF: no collective dependency
                emit_set(lambda c, u: kTl_sb[:, c, u * P:(u + 1) * P],
                         lambda u: vl_sb[:, u, :], MB * 4, "copy")

                for a in range(NQ):
                    kT_q = blk.tile([P, DC, NR * QTOK], BF16, name="kT_q")
                    v_q = blk.tile([P, NCH_R, D], BF16, name="v_q")
                    for j in range(NR):
                        d1 = nc.sync.dma_start(
                            kT_q[:, :, j * QTOK:(j + 1) * QTOK],
                            kv_all[a].ap()[bass_ds(kt_rv[j], KT_Q)]
                            .rearrange("(p c m) -> p c m", p=P, c=DC))
                        d2 = nc.sync.dma_start(
                            v_q[:, j * QT:(j + 1) * QT, :],
                            kv_all[a].ap()[bass_ds(v_rv[j], V_Q)]
                            .rearrange("(p t d) -> p t d", p=P, t=QT))
                        # dynamic-offset APs are not region-tracked against
                        # the collective's write; order them explicitly
                        for dd in (d1, d2):
                            bass_dep(dd.ins, cc_insts[a].ins, sync=True,
                                     reason="dyn kv read after gather")
                    emit_set(lambda c, u, kT_q=kT_q: kT_q[:, c, u * P:(u + 1) * P],
                             lambda u, v_q=v_q: v_q[:, u, :], NCH_R, "add")
                flush_pending()

                # ---- normalize and write out (per m-block, pipelined) ----
                rcpw = persist.tile([P, MB, 512], F32)
                scr = persist.tile([P, MB * 4 * 32], F32)
                h_dv = h_d.ap().rearrange("(t p) d -> p t d", p=P)
                for mb in range(MB):
                    for mt in range(4):
                        j = mb * 4 + mt
                        for x in range(4):
                            nc.vector.transpose(
                                scr[32 * x:32 * x + 32, j * 32:(j + 1) * 32],
                                den_acc[32 * x:32 * x + 32, mb,
                                        mt * P + 32 * x:mt * P + 32 * x + 32])
                        nc.vector.reciprocal(rcpw[:, mb, mt:mt + 1],
                                             scr[:, j * 32:j * 32 + 1])
                        nc.vector.tensor_scalar_mul(
                            h_acc[:, j, :], h_acc[:, j, :],
                            rcpw[:, mb, mt:mt + 1])
                        nc.sync.dma_start(h_dv[:, j, :], h_acc[:, j, :])

    nc.compile()
    return nc


_cache = {}


def kernel(z, Wq, bq, Wk, bk, Wv, bv):
    if "nc" not in _cache:
        _cache["nc"] = _build()
    nc = _cache["nc"]

    bf16 = ml_dtypes.bfloat16
    z, Wq, bq, Wk, bk, Wv, bv = (np.asarray(t) for t in
                                 (z, Wq, bq, Wk, bk, Wv, bv))
    z = np.ascontiguousarray(z, dtype=np.float32)
    zT = np.ascontiguousarray(z.T).astype(bf16)
    base = {
        "WqT": np.ascontiguousarray(Wq.T).astype(bf16),
        "WkT": np.ascontiguousarray(Wk.T).astype(bf16),
        "WvT": np.ascontiguousarray(Wv.T).astype(bf16),
        "bq": np.ascontiguousarray(bq, dtype=np.float32),
        "bk": np.ascontiguousarray(bk, dtype=np.float32),
        "bv": np.ascontiguousarray(bv).astype(bf16).reshape(1, D),
        "ones_row": np.ones((1, P), dtype=bf16),
        "ones_sq": np.ones((P, P), dtype=bf16),
    }
    in_maps = []
    for i in range(N_CORES):
        m = dict(base)
        m["zT_loc"] = np.ascontiguousarray(zT[:, i * M_LOC:(i + 1) * M_LOC])
        rem = [((i + 1 + j) % N_CORES) * KV_Q for j in range(N_CORES - 1)]
        m["offs"] = np.array([rem + [r + KT_Q for r in rem]], dtype=np.int32)
        in_maps.append(m)

    _cache["in_maps"] = in_maps
    res = run_bass_kernel_spmd(nc, in_maps, core_ids=list(range(N_CORES)))
    _cache["last_result"] = res
    return np.concatenate(
        [res.results[i]["h_out"] for i in range(N_CORES)], axis=0)

